# revision 11
# baseline (speedup 1.0000x reference)
"""CachedParamMgr cache-management step on 8 Trainium2 NeuronCores.

Exact reconstruction of the v3 configuration (measured 50312 ns):
13 chunks, queue == sub-shard, per-call to_reg immediates, DVE casts,
explicit mlp library load.  See kernel.py for the full derivation notes.
"""

from contextlib import ExitStack

import numpy as np

import concourse.bacc as bacc
import concourse.mybir as mybir
from concourse.bass_utils import run_bass_kernel_spmd
from concourse.library_config import mlp

N_EMB = 1_000_000
DIM = 128
N_CORES = 8
N_SUB = 4
ROWS_PER_SUB = N_EMB // (N_CORES * N_SUB)
ROWS_PER_CORE = N_EMB // N_CORES
CAP_FLOOR = 2176

_nc_cache: dict[int, object] = {}


def _split3(x):
    p = -(-x // (3 * 128)) * 128
    pieces = [p, p, x - 2 * p]
    assert all(q > 0 and q % 128 == 0 for q in pieces), (x, pieces)
    return pieces


def _chunks_for_cap(cap: int):
    return [[128] + _split3(cap - 128)] + [_split3(cap)] * (N_SUB - 1)


def _schedule(cap: int):
    per_sub = _chunks_for_cap(cap)
    sched = []
    max_rounds = max(len(c) for c in per_sub)
    for r in range(max_rounds):
        for s in range(N_SUB):
            if r < len(per_sub[s]):
                off = sum(per_sub[s][:r])
                sched.append((s, off, per_sub[s][r]))
    return sched


def _build_nc(cap: int):
    sched = _schedule(cap)
    n_chunks = len(sched)
    nc = bacc.Bacc("TRN2", target_bir_lowering=False, debug=False,
                   num_swdge_queues=4)
    table = nc.dram_tensor("table", [ROWS_PER_CORE, DIM],
                           mybir.dt.float32, kind="ExternalInput")
    idxs = nc.dram_tensor("idxs", [128, N_SUB * cap // 16],
                          mybir.dt.int16, kind="ExternalInput")
    out16 = nc.dram_tensor("out16", [128, N_SUB * cap],
                           mybir.dt.bfloat16, kind="ExternalOutput")

    with (
        nc.sbuf_tensor("dst", [128, N_SUB * cap], mybir.dt.float32) as dst,
        nc.sbuf_tensor("dst16", [128, N_SUB * cap], mybir.dt.bfloat16) as dst16,
        nc.sbuf_tensor("idx_sb", [128, N_SUB * cap // 16], mybir.dt.int16) as idx_sb,
        nc.semaphore("io") as io,
        nc.semaphore("vs") as vs,
        nc.semaphore("os0") as os0,
        nc.semaphore("os1") as os1,
        ExitStack() as stack,
        nc.Block() as block,
    ):
        gsems = [stack.enter_context(nc.semaphore(f"g{i}"))
                 for i in range(len(sched))]

        def chunk_dst(s, coff, size):
            a = s * cap + coff
            return a, a + size

        @block.gpsimd
        def _(gpsimd):
            gpsimd.load_library(mlp)
            gpsimd.wait_ge(io, 16)
            for i, (s, coff, size) in enumerate(sched):
                a, b = chunk_dst(s, coff, size)
                dst_ap = dst[:, a:b].rearrange("p (b e) -> p b e", e=DIM)
                gpsimd.dma_gather(
                    dst_ap,
                    table.ap()[s * ROWS_PER_SUB:(s + 1) * ROWS_PER_SUB, :],
                    idx_sb[:, (s * cap + coff) // 16:(s * cap + coff + size) // 16],
                    size, size, DIM,
                    single_packet=False,
                    queue_num=s,
                ).then_inc(gsems[i], 16)

        @block.vector
        def _(vector):
            for i, (s, coff, size) in enumerate(sched):
                a, b = chunk_dst(s, coff, size)
                vector.wait_ge(gsems[i], 16)
                vector.tensor_copy(dst16[:, a:b], dst[:, a:b]).then_inc(vs, 1)

        @block.sync
        def _(sync):
            sync.dma_start(idx_sb[:], idxs.ap()[:]).then_inc(io, 16)
            for i, (s, coff, size) in enumerate(sched):
                if i % 2:
                    continue
                a, b = chunk_dst(s, coff, size)
                sync.wait_ge(vs, i + 1)
                sync.dma_start(
                    out16.ap()[:, a:b], dst16[:, a:b],
                ).then_inc(os0, 16)
            sync.wait_ge(os0, 16 * ((n_chunks + 1) // 2))

        @block.scalar
        def _(scalar):
            for i, (s, coff, size) in enumerate(sched):
                if i % 2 == 0:
                    continue
                a, b = chunk_dst(s, coff, size)
                scalar.wait_ge(vs, i + 1)
                scalar.dma_start(
                    out16.ap()[:, a:b], dst16[:, a:b],
                ).then_inc(os1, 16)
            scalar.wait_ge(os1, 16 * (n_chunks // 2))

    nc.compile()
    return nc


def kernel(weight, cuda_cached_weight, cached_idx_map, inverted_cached_idx, ids,
           _profile=None):
    weight = np.asarray(weight)
    ids = np.asarray(ids)
    n_ids = ids.shape[0]

    ids64 = ids.astype(np.int64)
    sub_global = ids64 // ROWS_PER_SUB
    local = (ids64 % ROWS_PER_SUB).astype(np.int16)
    order = np.argsort(sub_global, kind="stable")
    counts = np.bincount(sub_global, minlength=N_CORES * N_SUB)
    starts = np.zeros(N_CORES * N_SUB + 1, dtype=np.int64)
    np.cumsum(counts, out=starts[1:])

    cap = max(CAP_FLOOR, -(-int(counts.max()) // 128) * 128)

    nc = _nc_cache.get(cap)
    if nc is None:
        nc = _nc_cache[cap] = _build_nc(cap)

    in_maps = []
    for c in range(N_CORES):
        idx_arr = np.zeros((128, N_SUB * cap // 16), dtype=np.int16)
        for s in range(N_SUB):
            gidx = c * N_SUB + s
            lst = local[order[starts[gidx]:starts[gidx + 1]]]
            padded = np.zeros(cap, dtype=np.int16)
            padded[:len(lst)] = lst
            idx_arr[:, s * cap // 16:(s + 1) * cap // 16] = np.tile(
                padded.reshape(cap // 16, 16).T, (8, 1))
        in_maps.append({
            "table": weight[c * ROWS_PER_CORE:(c + 1) * ROWS_PER_CORE],
            "idxs": idx_arr,
        })

    res = run_bass_kernel_spmd(
        nc, in_maps, core_ids=list(range(N_CORES)),
        **({"trace": True} if _profile is not None else {}),
    )
    if _profile is not None:
        _profile.append(res)

    out_full = np.empty((n_ids, DIM), dtype=np.float32)
    for c in range(N_CORES):
        core_out = np.asarray(res.results[c]["out16"]).astype(np.float32)
        for s in range(N_SUB):
            gidx = c * N_SUB + s
            cnt = counts[gidx]
            if cnt == 0:
                continue
            pos = order[starts[gidx]:starts[gidx + 1]]
            blk = core_out[:, s * cap:(s + 1) * cap].reshape(
                128, cap // 128, DIM)
            out_full[pos] = blk.transpose(1, 0, 2).reshape(cap, DIM)[:cnt]
    return out_full
